# revision 16
# baseline (speedup 1.0000x reference)
"""Multi-head attention + residual + LayerNorm on 8 Trainium2 NeuronCores.

Reference computation (B=2, S=2048, D=1024, H=16, HD=64):
    q = query @ Wq + bq ; k = key @ Wk + bk ; v = value @ Wv + bv   (per-head)
    scores = q k^T / sqrt(HD), masked (-inf where mask), softmax
    att = scores @ v ; out = att @ Wo + bo
    y = LayerNorm(query + out)   (std ddof=1, denom = std + 1e-6)

Sharding (v2):
  Launch 1: 8 cores = 2 batches x 4 head-groups (4 heads/core).
    Streamed structure: input DMAs are chunked so the first score matmul
    can issue ~8us in; QKV projections are emitted just-in-time,
    interleaved into the attention stream so the PE never drains.
    Attention runs in 4 passes (sq-group g in {0,1} x head-pair t in
    {0,1}); within a pass kk (sk chunks of 128) is the inner loop and
    PV accumulates into [65, 1024] PSUM tiles (ones-column of V gives
    softmax row-sums for free).  Row-sum normalization happens in-launch:
    reciprocal (DVE) -> partition_broadcast (GpSimd) -> multiply (GpSimd),
    so launch 2 needs no rowsum plumbing.  Output: normalized att^T.
  Launch 2: 8 cores = 2 batches x 4 seq-quarters (512 rows/core).
    out-proj (att^T is exactly the lhsT the matmul wants), residual
    (query + bo prefolded host-side), LayerNorm via Square-activation
    accumulate.
"""

import numpy as np
import ml_dtypes

import concourse.bass as bass
import concourse.tile as tile
from concourse import bacc, mybir
from concourse.bass_utils import run_bass_kernel_spmd

BF16 = ml_dtypes.bfloat16
F32 = np.float32
dt = mybir.dt

B, S, D, H, HD = 2, 2048, 1024, 16, 64
NCORES = 8
EPS = 1e-6
KC = D // 128  # 8 contraction chunks over D
SKC = S // 128  # 16 chunks of 128 over sk
SQR = S // 4  # 512 rows per core in launch 2

AF = mybir.ActivationFunctionType
ALU = mybir.AluOpType
AX = mybir.AxisListType

TRACE = False
LAST_EXEC_NS = []
_CACHE = {}
DEBUG_STASH = {}


def _emit_launch1(tc, qT, kT, vT, mcT, wq, wk, wv, bq, bk, bv, attT):
    nc = tc.nc
    from contextlib import ExitStack

    with ExitStack() as ctx:
        consts = ctx.enter_context(tc.tile_pool(name="consts", bufs=1))

        ones_bf = consts.tile([1, 512], dt.bfloat16)
        nc.vector.memset(ones_bf[:], 1.0)

        # biases: [128, 2] layout, col t = head-pair t's 128 dims
        bq_sb = consts.tile([1, 256], dt.bfloat16)
        nc.scalar.dma_start(bq_sb[:], bq.unsqueeze(0))
        bk_sb = consts.tile([1, 256], dt.bfloat16)
        nc.scalar.dma_start(bk_sb[:], bk.unsqueeze(0))
        bv_sb = consts.tile([1, 256], dt.bfloat16)
        nc.scalar.dma_start(bv_sb[:], bv.unsqueeze(0))

        wq_sb = consts.tile([128, KC, 256], dt.bfloat16)
        nc.scalar.dma_start(wq_sb[:], wq.rearrange("(c p) m -> p c m", p=128))
        wk_sb = consts.tile([128, KC, 256], dt.bfloat16)
        nc.scalar.dma_start(wk_sb[:], wk.rearrange("(c p) m -> p c m", p=128))
        wv_sb = consts.tile([128, KC, 256], dt.bfloat16)
        nc.scalar.dma_start(wv_sb[:], wv.rearrange("(c p) m -> p c m", p=128))

        # projected q^T / k^T: [128 partitions = pair dims, pair, S]
        proj = ctx.enter_context(tc.tile_pool(name="proj", bufs=1))
        qTp = proj.tile([128, 2, S], dt.bfloat16)
        kTp = proj.tile([128, 2, S], dt.bfloat16)
        # V with ones column per head: [sk-chunk, SKC, 4 heads, HD+1]
        vext = proj.tile([128, SKC, 4, HD + 1], dt.bfloat16)
        nc.vector.memset(vext[:], 1.0)

        # raw transposed inputs, streamed per (c, half-of-S)
        raw = ctx.enter_context(tc.tile_pool(name="raw", bufs=1))
        qT_sb = raw.tile([128, KC, S], dt.bfloat16)
        kT_sb = raw.tile([128, KC, S], dt.bfloat16)
        vT_sb = raw.tile([128, KC, S], dt.bfloat16)
        qTr = qT.rearrange("(c p) s -> p c s", p=128)
        kTr = kT.rearrange("(c p) s -> p c s", p=128)
        vTr = vT.rearrange("(c p) s -> p c s", p=128)
        for sh in range(2):
            sl = slice(sh * 1024, (sh + 1) * 1024)
            for c in range(KC):
                nc.sync.dma_start(kT_sb[:, c, sl], kTr[:, c, sl])
        for sh in range(2):
            sl = slice(sh * 1024, (sh + 1) * 1024)
            for c in range(KC):
                nc.gpsimd.dma_start(vT_sb[:, c, sl], vTr[:, c, sl])
            for c in range(KC):
                nc.gpsimd.dma_start(qT_sb[:, c, sl], qTr[:, c, sl])

        # mask tiles: [128, 8 kk, 1024 sq-group-cols], bufs=2, streamed per
        # (pass, half).  Issued on sync after the k raw chunks.
        mskp = ctx.enter_context(tc.tile_pool(name="mskp", bufs=2))
        mcTr = mcT.rearrange("(c p) s -> p c s", p=128)

        psum = ctx.enter_context(tc.tile_pool(name="psum", bufs=2, space="PSUM"))
        accp = ctx.enter_context(tc.tile_pool(name="accp", bufs=2, space="PSUM"))
        drp = ctx.enter_context(tc.tile_pool(name="drp", bufs=2, space="DRAM"))
        pp = ctx.enter_context(tc.tile_pool(name="pp", bufs=2))
        stp = ctx.enter_context(tc.tile_pool(name="stp", bufs=2))
        smalls = ctx.enter_context(tc.tile_pool(name="smalls", bufs=2))
        astp = ctx.enter_context(tc.tile_pool(name="astp", bufs=2))

        def _proj_qk(dst, w_sb, b_sb, src, t, sh):
            ps = psum.tile([128, 1024], dt.float32, tag="sp", name=f"pp{t}{sh}")
            for h2 in range(2):
                o = ps[:, h2 * 512 : (h2 + 1) * 512]
                sl = slice(sh * 1024 + h2 * 512, sh * 1024 + (h2 + 1) * 512)
                for c in range(KC):
                    nc.tensor.matmul(
                        o,
                        lhsT=w_sb[:, c, t * 128 : (t + 1) * 128],
                        rhs=src[:, c, sl],
                        start=(c == 0),
                        stop=False,
                    )
                nc.tensor.matmul(
                    o,
                    lhsT=b_sb[0:1, t * 128 : (t + 1) * 128],
                    rhs=ones_bf[0:1, 0:512],
                    start=False,
                    stop=True,
                )
            nc.vector.tensor_copy(dst[:, t, sh * 1024 : (sh + 1) * 1024], ps[:])

        def kproj(t, sh):
            _proj_qk(kTp, wk_sb, bk_sb, kT_sb, t, sh)

        def qproj(t, sh):
            _proj_qk(qTp, wq_sb, bq_sb, qT_sb, t, sh)

        def vproj(kk):
            ps = psum.tile([128, 1024], dt.float32, tag="sp", name=f"psv{kk}")
            vps = ps[:, 0:256]
            for c in range(KC):
                nc.tensor.matmul(
                    vps,
                    lhsT=vT_sb[:, c, kk * 128 : (kk + 1) * 128],
                    rhs=wv_sb[:, c, :],
                    start=(c == 0),
                    stop=False,
                )
            nc.tensor.matmul(
                vps, lhsT=ones_bf[0:1, 0:128], rhs=bv_sb[:], start=False, stop=True
            )
            nc.vector.tensor_copy(
                vext[:, kk, :, 0:HD],
                vps.rearrange("p (h d) -> p h d", h=4),
            )

        # prologue projections (min set for pass (g0, t0) kk0)
        kproj(0, 0)
        vproj(0)
        qproj(0, 0)

        # just-in-time projection thunks per pass: {step: [thunks]}.
        # vproj(kk) MUST be emitted before attention step kk of the first
        # pass (PV reads vext[kk]); extra projections ride along.
        jit00 = {kk: [(lambda kk=kk: vproj(kk))] for kk in range(1, SKC)}
        jit00[5].append(lambda: kproj(0, 1))
        jit00[9].append(lambda: kproj(1, 0))
        jit00[11].append(lambda: kproj(1, 1))
        jit00[13].append(lambda: qproj(1, 0))
        jit = {
            (0, 0): jit00,
            (0, 1): {
                2: [lambda: qproj(0, 1)],
                6: [lambda: qproj(1, 1)],
            },
        }

        def attention_pass(g, t):
            accs = [
                accp.tile([65, 1024], dt.float32, tag="acc", name=f"acc{g}{t}{hi}")
                for hi in range(2)
            ]
            thunks = jit.get((g, t), {})
            for half in range(4):
                mt = mskp.tile([128, 4, 1024], dt.bfloat16, tag="msk")
                for j in range(2):
                    nc.sync.dma_start(
                        mt[:, 2 * j : 2 * j + 2, :],
                        mcTr[
                            :,
                            half * 4 + 2 * j : half * 4 + 2 * j + 2,
                            g * 1024 : (g + 1) * 1024,
                        ],
                    )
                for kkl in range(4):
                    kk = half * 4 + kkl
                    for th in thunks.get(kk, []):
                        th()
                    for sqh in range(2):
                        # concurrent score pair: hi0 rows 0-63, hi1 rows 64-127
                        sp = psum.tile(
                            [128, 1024], dt.float32, tag="sp", name=f"sps{sqh}"
                        )
                        for hi in range(2):
                            nc.tensor.matmul(
                                sp[:, hi * 512 : (hi + 1) * 512],
                                lhsT=kTp[
                                    hi * 64 : (hi + 1) * 64,
                                    t,
                                    kk * 128 : (kk + 1) * 128,
                                ],
                                rhs=qTp[
                                    hi * 64 : (hi + 1) * 64,
                                    t,
                                    g * 1024 + sqh * 512 : g * 1024 + (sqh + 1) * 512,
                                ],
                                start=True,
                                stop=True,
                                tile_position=(hi * 64, 0),
                            )
                        p = pp.tile([128, 1024], dt.bfloat16, tag="p")
                        nc.scalar.activation(p[:], sp[:], AF.Exp, scale=0.125)
                        pm = pp.tile([128, 1024], dt.bfloat16, tag="pm")
                        nc.vector.tensor_mul(
                            pm[:].rearrange("p (h s) -> p h s", h=2),
                            p[:].rearrange("p (h s) -> p h s", h=2),
                            mt[:, kkl, sqh * 512 : (sqh + 1) * 512]
                            .unsqueeze(1)
                            .broadcast_to([128, 2, 512]),
                        )
                        for hi in range(2):
                            nc.tensor.matmul(
                                accs[hi][:, sqh * 512 : (sqh + 1) * 512],
                                lhsT=vext[:, kk, 2 * t + hi, :],
                                rhs=pm[:, hi * 512 : (hi + 1) * 512],
                                start=(kk == 0),
                                stop=(kk == SKC - 1),
                            )
            # drain: copy acc out of PSUM fast, then normalize.  The
            # partition-broadcast of the per-column reciprocal goes through
            # a DRAM bounce (DMA can replicate a row across partitions).
            ast = astp.tile([128, 1024], dt.bfloat16, tag="ast", name=f"ast{g}{t}")
            for hi in range(2):
                st = stp.tile([65, 1024], dt.float32, tag="st")
                nc.vector.tensor_copy(st[:], accs[hi][:])
                # reciprocal of the rowsum row: bounce through DRAM to
                # spread the [1, 1024] row across 128 partitions (a [1, N]
                # DVE reciprocal runs on one lane: ~6.5us; [128, 8] is ~free)
                rsd = drp.tile([1, 1024], dt.float32, tag="rsd")
                nc.gpsimd.dma_start(rsd[:], st[64:65, :])
                rs128 = smalls.tile([128, 8], dt.float32, tag="rs128")
                nc.gpsimd.dma_start(
                    rs128[:], rsd.rearrange("a (p j) -> (a p) j", p=128)
                )
                rc128 = smalls.tile([128, 8], dt.float32, tag="rc128")
                nc.vector.reciprocal(rc128[:], rs128[:])
                rcd = drp.tile([128, 8], dt.float32, tag="rcd")
                nc.gpsimd.dma_start(rcd[:], rc128[:])
                rb = smalls.tile([64, 1024], dt.float32, tag="rb")
                nc.gpsimd.dma_start(
                    rb[:],
                    rcd.rearrange("p j -> (p j)").unsqueeze(0).broadcast_to(
                        [64, 1024]
                    ),
                )
                nc.vector.tensor_mul(
                    ast[hi * 64 : (hi + 1) * 64, :], st[0:64, :], rb[:]
                )
            nc.gpsimd.dma_start(
                attT[t * 128 : (t + 1) * 128, g * 1024 : (g + 1) * 1024], ast[:]
            )

        attention_pass(0, 0)
        attention_pass(0, 1)
        attention_pass(1, 0)
        attention_pass(1, 1)


def _emit_launch2(tc, aT, wo, resid, gamma, beta, out):
    """out-proj + residual + LayerNorm for one seq quarter (512 rows).

    aT: normalized att^T [D, SQR] bf16.  resid already includes bo.
    """
    nc = tc.nc
    from contextlib import ExitStack

    MC = SQR // 128  # 4 chunks of 128 rows

    with ExitStack() as ctx:
        consts = ctx.enter_context(tc.tile_pool(name="consts", bufs=1))
        work = ctx.enter_context(tc.tile_pool(name="work", bufs=3))
        stats = ctx.enter_context(tc.tile_pool(name="stats", bufs=8))
        psp = ctx.enter_context(tc.tile_pool(name="psp", bufs=4, space="PSUM"))

        aT_sb = consts.tile([128, KC, SQR], dt.bfloat16)
        wo_sb = consts.tile([128, KC, D], dt.bfloat16)
        res_sb = consts.tile([128, MC, D], dt.float32)
        aTr = aT.rearrange("(c p) s -> p c s", p=128)
        wor = wo.rearrange("(c p) m -> p c m", p=128)
        resr = resid.rearrange("(m p) d -> p m d", p=128)
        for c in range(KC):
            nc.sync.dma_start(aT_sb[:, c, :], aTr[:, c, :])
            nc.scalar.dma_start(wo_sb[:, c, :], wor[:, c, :])
        for m in range(MC):
            nc.gpsimd.dma_start(res_sb[:, m, :], resr[:, m, :])
        gam = consts.tile([128, D], dt.float32)
        nc.sync.dma_start(gam[:], gamma.unsqueeze(0).broadcast_to([128, D]))
        bet = consts.tile([128, D], dt.float32)
        nc.sync.dma_start(bet[:], beta.unsqueeze(0).broadcast_to([128, D]))

        for m in range(MC):
            ps = psp.tile([128, 1024], dt.float32, tag="ps")
            for dh in range(2):
                for c in range(KC):
                    nc.tensor.matmul(
                        ps[:, dh * 512 : (dh + 1) * 512],
                        lhsT=aT_sb[:, c, m * 128 : (m + 1) * 128],
                        rhs=wo_sb[:, c, dh * 512 : (dh + 1) * 512],
                        start=(c == 0),
                        stop=(c == KC - 1),
                    )
            x = work.tile([128, D], dt.float32, tag="x")
            nc.vector.tensor_add(x[:], ps[:], res_sb[:, m, :])
            # LayerNorm over D (ddof=1, denom std+eps)
            mn = stats.tile([128, 1], dt.float32, tag="mn")
            nc.vector.reduce_sum(mn[:], x[:], axis=AX.X)
            nc.vector.tensor_scalar_mul(mn[:], mn[:], -1.0 / D)
            scr = work.tile([128, D], dt.float32, tag="scr")
            vs = stats.tile([128, 1], dt.float32, tag="vs")
            nc.scalar.activation(scr[:], x[:], AF.Square, bias=mn[:], accum_out=vs[:])
            sd = stats.tile([128, 1], dt.float32, tag="sd")
            nc.scalar.activation(sd[:], vs[:], AF.Sqrt, scale=1.0 / (D - 1))
            nc.vector.tensor_scalar_add(sd[:], sd[:], EPS)
            rc = stats.tile([128, 1], dt.float32, tag="rc")
            nc.vector.reciprocal(rc[:], sd[:])
            # xc = (x - mean) * rstd ; y = xc * gamma + beta
            xc = work.tile([128, D], dt.float32, tag="xc")
            nc.vector.tensor_scalar(
                out=xc[:],
                in0=x[:],
                scalar1=mn[:],
                scalar2=rc[:],
                op0=ALU.add,
                op1=ALU.mult,
            )
            t2 = work.tile([128, D], dt.float32, tag="t2")
            nc.vector.tensor_mul(t2[:], xc[:], gam[:])
            yo = work.tile([128, D], dt.float32, tag="yo")
            nc.vector.tensor_add(yo[:], t2[:], bet[:])
            nc.gpsimd.dma_start(out.rearrange("(m p) d -> p m d", p=128)[:, m, :], yo[:])


def _build_launch1():
    nc = bacc.Bacc("TRN2", debug=False, enable_asserts=False)
    qT = nc.dram_tensor("qT", [D, S], dt.bfloat16, kind="ExternalInput").ap()
    kT = nc.dram_tensor("kT", [D, S], dt.bfloat16, kind="ExternalInput").ap()
    vT = nc.dram_tensor("vT", [D, S], dt.bfloat16, kind="ExternalInput").ap()
    mcT = nc.dram_tensor("mcT", [S, S], dt.bfloat16, kind="ExternalInput").ap()
    wq = nc.dram_tensor("wq", [D, 256], dt.bfloat16, kind="ExternalInput").ap()
    wk = nc.dram_tensor("wk", [D, 256], dt.bfloat16, kind="ExternalInput").ap()
    wv = nc.dram_tensor("wv", [D, 256], dt.bfloat16, kind="ExternalInput").ap()
    bq = nc.dram_tensor("bq", [256], dt.bfloat16, kind="ExternalInput").ap()
    bk = nc.dram_tensor("bk", [256], dt.bfloat16, kind="ExternalInput").ap()
    bv = nc.dram_tensor("bv", [256], dt.bfloat16, kind="ExternalInput").ap()
    attT = nc.dram_tensor("attT", [256, S], dt.bfloat16, kind="ExternalOutput").ap()
    with tile.TileContext(nc) as tc:
        _emit_launch1(tc, qT, kT, vT, mcT, wq, wk, wv, bq, bk, bv, attT)
    nc.compile()
    return nc


def _build_launch2():
    nc = bacc.Bacc("TRN2", debug=False, enable_asserts=False)
    aT = nc.dram_tensor("aT", [D, SQR], dt.bfloat16, kind="ExternalInput").ap()
    wo = nc.dram_tensor("wo", [D, D], dt.bfloat16, kind="ExternalInput").ap()
    resid = nc.dram_tensor("resid", [SQR, D], dt.float32, kind="ExternalInput").ap()
    gamma = nc.dram_tensor("gamma", [D], dt.float32, kind="ExternalInput").ap()
    beta = nc.dram_tensor("beta", [D], dt.float32, kind="ExternalInput").ap()
    out = nc.dram_tensor("out", [SQR, D], dt.float32, kind="ExternalOutput").ap()
    with tile.TileContext(nc) as tc:
        _emit_launch2(tc, aT, wo, resid, gamma, beta, out)
    nc.compile()
    return nc


def _get(name):
    if name not in _CACHE:
        _CACHE[name] = _build_launch1() if name == "l1" else _build_launch2()
    return _CACHE[name]


def kernel(query, key, value, mask, Wq, bq, Wk, bk, Wv, bv, Wo, bo, gamma, beta):
    global LAST_EXEC_NS
    LAST_EXEC_NS = []
    query = np.asarray(query, dtype=F32)
    key = np.asarray(key, dtype=F32)
    value = np.asarray(value, dtype=F32)
    mask = np.asarray(mask)
    Wq, Wk, Wv, Wo = (np.asarray(a, dtype=F32) for a in (Wq, Wk, Wv, Wo))
    bq, bk, bv, bo = (np.asarray(a, dtype=F32) for a in (bq, bk, bv, bo))
    gamma = np.asarray(gamma, dtype=F32)
    beta = np.asarray(beta, dtype=F32)

    # ---- launch 1: attention, sharded (batch x 4-head-group) ----
    qTl = [np.ascontiguousarray(query[b].T.astype(BF16)) for b in range(B)]
    kTl = [np.ascontiguousarray(key[b].T.astype(BF16)) for b in range(B)]
    vTl = [np.ascontiguousarray(value[b].T.astype(BF16)) for b in range(B)]
    mcT = [np.ascontiguousarray((~mask[b]).T.astype(BF16)) for b in range(B)]

    in_maps1 = []
    for c in range(NCORES):
        b, g = c // 4, c % 4
        sl = slice(g * 256, (g + 1) * 256)
        in_maps1.append(
            {
                "qT": qTl[b],
                "kT": kTl[b],
                "vT": vTl[b],
                "mcT": mcT[b],
                "wq": np.ascontiguousarray(Wq[:, sl].astype(BF16)),
                "wk": np.ascontiguousarray(Wk[:, sl].astype(BF16)),
                "wv": np.ascontiguousarray(Wv[:, sl].astype(BF16)),
                "bq": np.ascontiguousarray(bq[sl].astype(BF16)),
                "bk": np.ascontiguousarray(bk[sl].astype(BF16)),
                "bv": np.ascontiguousarray(bv[sl].astype(BF16)),
            }
        )
    nc1 = _get("l1")
    r1 = run_bass_kernel_spmd(nc1, in_maps1, core_ids=list(range(NCORES)), trace=TRACE)
    if TRACE:
        LAST_EXEC_NS.append(r1.exec_time_ns)

    # assemble normalized att^T per batch: [1024, S]
    attT_full = [
        np.concatenate([r1.results[b * 4 + g]["attT"] for g in range(4)], axis=0)
        for b in range(B)
    ]
    DEBUG_STASH["attT"] = attT_full

    # ---- launch 2: out-proj + residual + LayerNorm ----
    wo_bf = np.ascontiguousarray(Wo.astype(BF16))
    in_maps2 = []
    for c in range(NCORES):
        b, q = c // 4, c % 4
        sl = slice(q * SQR, (q + 1) * SQR)
        in_maps2.append(
            {
                "aT": np.ascontiguousarray(attT_full[b][:, sl]),
                "wo": wo_bf,
                "resid": np.ascontiguousarray(query[b, sl, :] + bo[None, :]),
                "gamma": gamma,
                "beta": beta,
            }
        )
    nc2 = _get("l2")
    r2 = run_bass_kernel_spmd(nc2, in_maps2, core_ids=list(range(NCORES)), trace=TRACE)
    if TRACE:
        LAST_EXEC_NS.append(r2.exec_time_ns)

    out = np.empty((B, S, D), dtype=F32)
    for c in range(NCORES):
        b, q = c // 4, c % 4
        out[b, q * SQR : (q + 1) * SQR, :] = r2.results[c]["out"]
    return out
